# revision 18
# baseline (speedup 1.0000x reference)
"""Trainium2 Bass kernel for batched softmax-attention readout:

    out[b] = softmax(S[b], axis=-1) @ U[b]

Shapes (hardcoded): S [B=128, T=2048, J=128] f32, U [B=128, J=128, d=512] f32,
out [B=128, T=2048, d=512] f32.

Sharding: batch dim B split across 8 NeuronCores (16 batches/core), fully
data-parallel (softmax and the A@U matmul are batch-local; no collectives).

The kernel is HBM-DMA bound, so all device I/O is bf16 (S/U downcast on host,
out upcast on host): 44MB/core vs 88MB in f32.  Measured end-to-end rel err
~8e-3 vs the 2e-2 gate (softmax needs no max-subtraction: |S| <~ 6).

Per-core pipeline, per batch b, T split into C=16 chunks of 128 rows
(row t = p*16 + c so S loads / out stores are 4KB-contiguous per partition):
  1. DMA S[b] (bf16) -> SBUF [128p, 16c, 128j]; U[b] (bf16) -> [128j, 512d]
  2. TensorE: per group of TG=4 chunks, 4 transposes of RAW S chunks into one
     PSUM bank [j, 4, t] (bf16 passthrough, exact)
  3. ScalarE: act-exp copyback E^T = exp(S^T): PSUM f32 -> SBUF bf16 lhsT
     layout.  Fuses exp with the mandatory PSUM evacuation (saves a full
     4.2M-elem/core exp pass vs exp-then-transpose).
  4. TensorE: r-matmul ps_r[:, c] = E_c^T.T @ ones  (N=1, partition-reduce)
     -- replaces a VectorE reduce_sum over 4.2M elems/core.
  5. VectorE: rinv[:, g] = 1 / ps_r[:, g]  (per group of 4)
  6. TensorE: out_psum[t, d] = E_c^T.T @ U[b]  (bf16, N=512)
  7. ScalarE/VectorE: o_sb = out_psum * rinv[:, c] -> bf16 (fused normalize +
     mandatory PSUM->SBUF evacuation, split across both engines)
  8. DMA out chunk groups -> HBM (bf16, 8KB/partition runs)

Engine budget per core (model): DMA ~140us measured floor for this access
pattern (44MB, ~315GB/s effective with all 8 cores active), ScalarE ~118us,
VectorE ~93us, TensorE ~110us -> DMA-bound.

DMA routing: input loads trigger from the gpsimd (SWDGE) queue, output
stores from the sync (HWDGE) queue.  On a single queue the store triggers
(which wait on evac semaphores) head-of-line-block the next batch's loads;
splitting queues removed that stall (~24us).  Input pools are 3-4 deep:
with u bufs=2 the U-load trigger for batch b waits on batch b-2's last
matmul and blocks the S-load behind it in the queue FIFO (~4us).

HW slope-timed (8 cores, paired repeat=1/129): ~138us vs 304us for the
f32 exp-then-transpose baseline.
"""

import sys

sys.path.insert(0, "/opt/trn_rl_repo")

from contextlib import ExitStack

import numpy as np
import ml_dtypes

import concourse.bass as bass
import concourse.mybir as mybir
import concourse.tile as tile
from concourse import bacc
from concourse.bass_utils import run_bass_kernel_spmd
from concourse.masks import make_identity

# Problem shapes
B, T, J, D = 128, 2048, 128, 512
N_CORES = 8
BPC = B // N_CORES  # batches per core
P = 128
C = T // P  # T-chunks per batch

# Tuning knobs
OG = 8  # out chunks per output DMA (OG*4KB/partition contiguous)
S_SPLIT = 2  # input-S DMAs per batch
ACT_EVAC = 7  # how many of the 16 out-evacs go to ScalarE (rest VectorE)
TG = 4  # chunks per transpose group (one PSUM bank)
LOOKAHEAD = 2  # transpose groups in flight ahead of matmuls
IN_DMA = "gpsimd"  # 'sync' | 'scalar' | 'gpsimd'
OUT_DMA = "sync"  # 'sync' | 'scalar' | 'gpsimd'
LAYOUT = "pc"  # 'pc' | 'h2' | 'h4' (split-T tiling, sequential HBM runs)
BUFS = dict(s=4, u=3, et=4, o=4, pst=3, pso=4, psr=1)

F32 = mybir.dt.float32
BF16 = mybir.dt.bfloat16
NP_BF16 = ml_dtypes.bfloat16


def build_nc(repeat=1, og=None, s_split=None, act_evac=None, bufs=None,
             tg=None, lookahead=None, in_dma=None, out_dma=None,
             layout=None, skip_out_dma=False, skip_in_dma=False,
             dma_only=False):
    og = OG if og is None else og
    s_split = S_SPLIT if s_split is None else s_split
    act_evac = ACT_EVAC if act_evac is None else act_evac
    tg = TG if tg is None else tg
    lookahead = LOOKAHEAD if lookahead is None else lookahead
    in_dma = IN_DMA if in_dma is None else in_dma
    out_dma = OUT_DMA if out_dma is None else out_dma
    layout = LAYOUT if layout is None else layout
    bufs = dict(BUFS, **(bufs or {}))
    # 'pc': row t = p*16 + c (chunk strided across HBM).  'h2': T in two
    # halves, row t = h*1024 + p*8 + c -- S loads and O stores become fully
    # sequential HBM runs (2KB/8KB per partition, no stride gaps) while
    # keeping half-batch dependency granularity.
    NH = {"pc": 1, "h2": 2, "h4": 4}[layout]
    CH = C // NH
    assert s_split % NH == 0 and og <= CH

    # Which of the C out-evacs run on ScalarE (rest on VectorE).  When the
    # out DMA triggers from the scalar queue, pin the LAST chunk of each
    # og-group to ScalarE so the trigger instruction never waits on a DVE
    # semaphore (no head-of-line blocking in the Act stream).
    act_set = set()
    if out_dma == "scalar":
        act_set.update(c for c in range(C) if c % og == og - 1)
    elif out_dma == "both":
        # odd og-groups trigger from the scalar queue; pin their last evac
        # to ScalarE so the trigger never waits on a DVE semaphore.
        act_set.update(
            c for c in range(C) if c % og == og - 1 and (c // og) % 2 == 1
        )
    i = 0
    while len(act_set) < act_evac and i < C:
        cand = (i * C) // max(act_evac, 1)
        if cand not in act_set:
            act_set.add(cand)
        i += 1
    # pad arbitrarily if the spread collided too much
    c = 0
    while len(act_set) < act_evac:
        if c not in act_set:
            act_set.add(c)
        c += 1
    nc = bacc.Bacc(
        "TRN2", target_bir_lowering=False, debug=False, num_devices=N_CORES
    )
    S = nc.dram_tensor("S", [BPC, T, J], BF16, kind="ExternalInput").ap()
    U = nc.dram_tensor("U", [BPC, J, D], BF16, kind="ExternalInput").ap()
    O = nc.dram_tensor("O", [BPC, T, D], BF16, kind="ExternalOutput").ap()

    def dma_eng(which):
        return {"sync": nc.sync, "scalar": nc.scalar, "gpsimd": nc.gpsimd}[which]

    in_eng = dma_eng(in_dma)

    def out_eng(og_g):
        if out_dma == "both":
            return nc.scalar if og_g % 2 == 1 else nc.sync
        return dma_eng(out_dma)

    with tile.TileContext(nc) as tc, ExitStack() as ctx:
        consts = ctx.enter_context(tc.tile_pool(name="consts", bufs=1))
        s_pool = ctx.enter_context(tc.tile_pool(name="s", bufs=bufs["s"]))
        u_pool = ctx.enter_context(tc.tile_pool(name="u", bufs=bufs["u"]))
        et_pool = ctx.enter_context(tc.tile_pool(name="et", bufs=bufs["et"]))
        o_pool = ctx.enter_context(tc.tile_pool(name="o", bufs=bufs["o"]))
        st_pool = ctx.enter_context(tc.tile_pool(name="stats", bufs=2))
        pst = ctx.enter_context(tc.tile_pool(name="pst", bufs=bufs["pst"], space="PSUM"))
        pso = ctx.enter_context(tc.tile_pool(name="pso", bufs=bufs["pso"], space="PSUM"))
        psr = ctx.enter_context(tc.tile_pool(name="psr", bufs=bufs["psr"], space="PSUM"))

        ident = consts.tile([P, P], BF16)
        make_identity(nc, ident)
        ones = consts.tile([P, 1], BF16)
        nc.vector.memset(ones[:], 1.0)

        if dma_only:
            o_fake = consts.tile([P, og, D], BF16)
            nc.vector.memset(o_fake[:], 0.0)

        loop_ctx = tc.For_i(0, repeat, 1) if repeat > 1 else None
        if loop_ctx is not None:
            ctx.enter_context(loop_ctx)

        NG = C // tg

        for b in range(BPC):
            s_halves = [
                S[b].rearrange("(h t) j -> h t j", h=NH)[h].rearrange(
                    "(p c) j -> p c j", c=CH
                )
                for h in range(NH)
            ]
            o_halves = [
                O[b].rearrange("(h t) d -> h t d", h=NH)[h].rearrange(
                    "(p c) d -> p c d", c=CH
                )
                for h in range(NH)
            ]

            def s_src_ap(a, bnd):
                h = a // CH
                return s_halves[h][:, a - h * CH : bnd - h * CH, :]

            def o_dst_ap(a, bnd):
                h = a // CH
                return o_halves[h][:, a - h * CH : bnd - h * CH, :]

            if dma_only:
                s_sb = s_pool.tile([P, C, J], BF16)
                for ss in range(s_split):
                    cs = C // s_split
                    in_eng.dma_start(
                        s_sb[:, ss * cs : (ss + 1) * cs, :],
                        s_src_ap(ss * cs, (ss + 1) * cs),
                    )
                u_sb = u_pool.tile([P, D], BF16)
                in_eng.dma_start(u_sb[:], U[b])
                for og_g in range(C // og):
                    out_eng(og_g).dma_start(
                        o_dst_ap(og_g * og, (og_g + 1) * og), o_fake[:]
                    )
                continue
            # --- loads ---
            s_sb = s_pool.tile([P, C, J], BF16)
            for ss in range(s_split):
                cs = C // s_split
                if not skip_in_dma:
                    in_eng.dma_start(
                        s_sb[:, ss * cs : (ss + 1) * cs, :],
                        s_src_ap(ss * cs, (ss + 1) * cs),
                    )
            if skip_in_dma:
                nc.vector.memset(s_sb[:, 0:1, :], 0.1)
            u_sb = u_pool.tile([P, D], BF16)
            if not skip_in_dma:
                in_eng.dma_start(u_sb[:], U[b])

            ps_r = psr.tile([P, C], F32, tag="ps_r", name=f"ps_r_{b}")
            rinv = st_pool.tile([P, C], F32, tag="rinv", name=f"rinv_{b}")

            et_sb = [None] * NG
            o_sb = [None] * (C // og)

            def do_transposes(g):
                # raw-S transposes (bf16 passthrough) into one PSUM bank,
                # then ONE act-exp copyback: E^T = exp(S^T) -> bf16 lhsT.
                et_ps = pst.tile([P, tg, P], BF16, tag="et_ps", name=f"et_ps_{b}_{g}")
                for k in range(tg):
                    nc.tensor.transpose(
                        et_ps[:, k, :], s_sb[:, g * tg + k, :], ident[:]
                    )
                et_sb[g] = et_pool.tile(
                    [P, tg, P], BF16, tag="et_sb", name=f"et_sb_{b}_{g}"
                )
                nc.scalar.activation(
                    et_sb[g][:], et_ps[:], mybir.ActivationFunctionType.Exp
                )

            def do_rmms(g):
                # softmax denominators via N=1 matmul against ones (PE
                # partition-reduce), one PSUM column per chunk; then recip.
                for k in range(tg):
                    c = g * tg + k
                    nc.tensor.matmul(
                        ps_r[:, c : c + 1], et_sb[g][:, k, :], ones[:],
                        start=True, stop=True,
                    )
                nc.vector.reciprocal(
                    rinv[:, g * tg : (g + 1) * tg],
                    ps_r[:, g * tg : (g + 1) * tg],
                )

            def do_matmul(c):
                o_ps = pso.tile([P, D], F32, tag="o_ps", name=f"o_ps_{b}_{c}")
                g, k = divmod(c, tg)
                nc.tensor.matmul(
                    o_ps[:], et_sb[g][:, k, :], u_sb[:], start=True, stop=True
                )
                og_g, gi = divmod(c, og)
                if gi == 0:
                    o_sb[og_g] = o_pool.tile(
                        [P, og, D], BF16, tag="o_sb", name=f"o_sb_{b}_{c}"
                    )
                if c in act_set:
                    nc.scalar.mul(o_sb[og_g][:, gi, :], o_ps[:], rinv[:, c : c + 1])
                else:
                    nc.vector.tensor_scalar_mul(
                        o_sb[og_g][:, gi, :], o_ps[:], rinv[:, c : c + 1]
                    )
                if gi == og - 1 and not skip_out_dma:
                    out_eng(og_g).dma_start(
                        o_dst_ap(og_g * og, (og_g + 1) * og), o_sb[og_g][:]
                    )

            # software pipeline: transposes run LOOKAHEAD groups ahead so the
            # act-exp latency hides behind PE matmuls of earlier groups.
            la = min(lookahead, NG)
            for g in range(la):
                do_transposes(g)
            for g in range(NG):
                if g + la < NG:
                    do_transposes(g + la)
                do_rmms(g)
                for k in range(tg):
                    do_matmul(g * tg + k)

    nc.compile()
    return nc


_NC_CACHE = None


def _get_nc():
    global _NC_CACHE
    if _NC_CACHE is None:
        _NC_CACHE = build_nc()
    return _NC_CACHE


def make_in_maps(U, S):
    U = np.asarray(U).astype(NP_BF16)
    S = np.asarray(S).astype(NP_BF16)
    return [
        {
            "S": np.ascontiguousarray(S[i * BPC : (i + 1) * BPC]),
            "U": np.ascontiguousarray(U[i * BPC : (i + 1) * BPC]),
        }
        for i in range(N_CORES)
    ]


def kernel(U, S):
    nc = _get_nc()
    in_maps = make_in_maps(U, S)
    try:
        res = run_bass_kernel_spmd(nc, in_maps, core_ids=list(range(N_CORES)))
    except Exception:
        # transient device/runtime hiccup: retry once
        res = run_bass_kernel_spmd(nc, in_maps, core_ids=list(range(N_CORES)))
    out = np.concatenate(
        [np.asarray(res.results[i]["O"]) for i in range(N_CORES)], axis=0
    ).astype(np.float32)
    return out


# revision 20
# speedup vs baseline: 1.0559x; 1.0559x over previous
"""Trainium2 Bass kernel for batched softmax-attention readout:

    out[b] = softmax(S[b], axis=-1) @ U[b]

Shapes (hardcoded): S [B=128, T=2048, J=128] f32, U [B=128, J=128, d=512] f32,
out [B=128, T=2048, d=512] f32.

Sharding: batch dim B split across 8 NeuronCores (16 batches/core), fully
data-parallel (softmax and the A@U matmul are batch-local; no collectives).

The kernel is HBM-DMA bound, so all device I/O is bf16 (S/U downcast on host,
out upcast on host): 44MB/core vs 88MB in f32.  Measured end-to-end rel err
~8e-3 vs the 2e-2 gate (softmax needs no max-subtraction: |S| <~ 6).

Per-core pipeline, per batch b, T split into C=16 chunks of 128 rows.
Row mapping ('h2'): t = h*1024 + p*8 + c -- each half's S load and out store
is a fully SEQUENTIAL HBM run (no stride gaps), worth ~3us/iter over the
strided t = p*16 + c mapping (drift-controlled paired A/B; pure-DMA floor
128.6us vs 139.7us):
  1. DMA S[b] (bf16) -> SBUF [128p, 16c, 128j]; U[b] (bf16) -> [128j, 512d]
  2. TensorE: per group of TG=4 chunks, 4 transposes of RAW S chunks into one
     PSUM bank [j, 4, t] (bf16 passthrough, exact)
  3. ScalarE: act-exp copyback E^T = exp(S^T): PSUM f32 -> SBUF bf16 lhsT
     layout.  Fuses exp with the mandatory PSUM evacuation (saves a full
     4.2M-elem/core exp pass vs exp-then-transpose).
  4. TensorE: r-matmul ps_r[:, c] = E_c^T.T @ ones  (N=1, partition-reduce)
     -- replaces a VectorE reduce_sum over 4.2M elems/core.
  5. VectorE: rinv[:, g] = 1 / ps_r[:, g]  (per group of 4)
  6. TensorE: out_psum[t, d] = E_c^T.T @ U[b]  (bf16, N=512)
  7. ScalarE/VectorE: o_sb = out_psum * rinv[:, c] -> bf16 (fused normalize +
     mandatory PSUM->SBUF evacuation, split across both engines)
  8. DMA out chunk groups -> HBM (bf16, 8KB/partition runs)

Engine budget per core (model): DMA ~140us measured floor for this access
pattern (44MB, ~315GB/s effective with all 8 cores active), ScalarE ~118us,
VectorE ~93us, TensorE ~110us -> DMA-bound.

DMA routing: input loads trigger from the gpsimd (SWDGE) queue, output
stores from the sync (HWDGE) queue.  On a single queue the store triggers
(which wait on evac semaphores) head-of-line-block the next batch's loads;
splitting queues removed that stall (~24us).  Input pools are 3-4 deep:
with u bufs=2 the U-load trigger for batch b waits on batch b-2's last
matmul and blocks the S-load behind it in the queue FIFO (~4us).

HW slope-timed (8 cores, paired repeat=1/129): ~138us vs 304us for the
f32 exp-then-transpose baseline.
"""

import sys

sys.path.insert(0, "/opt/trn_rl_repo")

from contextlib import ExitStack

import numpy as np
import ml_dtypes

import concourse.bass as bass
import concourse.mybir as mybir
import concourse.tile as tile
from concourse import bacc
from concourse.bass_utils import run_bass_kernel_spmd
from concourse.masks import make_identity

# Problem shapes
B, T, J, D = 128, 2048, 128, 512
N_CORES = 8
BPC = B // N_CORES  # batches per core
P = 128
C = T // P  # T-chunks per batch

# Tuning knobs
OG = 8  # out chunks per output DMA (OG*4KB/partition contiguous)
S_SPLIT = 2  # input-S DMAs per batch
ACT_EVAC = 7  # how many of the 16 out-evacs go to ScalarE (rest VectorE)
TG = 4  # chunks per transpose group (one PSUM bank)
LOOKAHEAD = 2  # transpose groups in flight ahead of matmuls
IN_DMA = "gpsimd"  # 'sync' | 'scalar' | 'gpsimd'
OUT_DMA = "sync"  # 'sync' | 'scalar' | 'gpsimd'
LAYOUT = "h2"  # 'pc' | 'h2' | 'h4' (split-T tiling, sequential HBM runs)
BUFS = dict(s=4, u=3, et=4, o=4, pst=3, pso=4, psr=1)

F32 = mybir.dt.float32
BF16 = mybir.dt.bfloat16
NP_BF16 = ml_dtypes.bfloat16


def build_nc(repeat=1, og=None, s_split=None, act_evac=None, bufs=None,
             tg=None, lookahead=None, in_dma=None, out_dma=None,
             layout=None, pin_last=False, skip_out_dma=False,
             skip_in_dma=False, dma_only=False):
    og = OG if og is None else og
    s_split = S_SPLIT if s_split is None else s_split
    act_evac = ACT_EVAC if act_evac is None else act_evac
    tg = TG if tg is None else tg
    lookahead = LOOKAHEAD if lookahead is None else lookahead
    in_dma = IN_DMA if in_dma is None else in_dma
    out_dma = OUT_DMA if out_dma is None else out_dma
    layout = LAYOUT if layout is None else layout
    bufs = dict(BUFS, **(bufs or {}))
    # 'pc': row t = p*16 + c (chunk strided across HBM).  'h2': T in two
    # halves, row t = h*1024 + p*8 + c -- S loads and O stores become fully
    # sequential HBM runs (2KB/8KB per partition, no stride gaps) while
    # keeping half-batch dependency granularity.
    NH = {"pc": 1, "h2": 2, "h4": 4}[layout]
    CH = C // NH
    assert s_split % NH == 0 and og <= CH

    # Which of the C out-evacs run on ScalarE (rest on VectorE).  When the
    # out DMA triggers from the scalar queue, pin the LAST chunk of each
    # og-group to ScalarE so the trigger instruction never waits on a DVE
    # semaphore (no head-of-line blocking in the Act stream).
    act_set = set()
    if pin_last:
        # store-gating chunks: the out-DMA trigger waits on the LAST evac of
        # each og-group; run those on ScalarE (faster per-op, usually ahead)
        act_set.update(c for c in range(C) if c % og == og - 1)
    if out_dma == "scalar":
        act_set.update(c for c in range(C) if c % og == og - 1)
    elif out_dma == "both":
        # odd og-groups trigger from the scalar queue; pin their last evac
        # to ScalarE so the trigger never waits on a DVE semaphore.
        act_set.update(
            c for c in range(C) if c % og == og - 1 and (c // og) % 2 == 1
        )
    i = 0
    while len(act_set) < act_evac and i < C:
        cand = (i * C) // max(act_evac, 1)
        if cand not in act_set:
            act_set.add(cand)
        i += 1
    # pad arbitrarily if the spread collided too much
    c = 0
    while len(act_set) < act_evac:
        if c not in act_set:
            act_set.add(c)
        c += 1
    nc = bacc.Bacc(
        "TRN2", target_bir_lowering=False, debug=False, num_devices=N_CORES
    )
    S = nc.dram_tensor("S", [BPC, T, J], BF16, kind="ExternalInput").ap()
    U = nc.dram_tensor("U", [BPC, J, D], BF16, kind="ExternalInput").ap()
    O = nc.dram_tensor("O", [BPC, T, D], BF16, kind="ExternalOutput").ap()

    def dma_eng(which):
        return {"sync": nc.sync, "scalar": nc.scalar, "gpsimd": nc.gpsimd}[which]

    in_eng = dma_eng(in_dma)

    def out_eng(og_g):
        if out_dma == "both":
            return nc.scalar if og_g % 2 == 1 else nc.sync
        return dma_eng(out_dma)

    with tile.TileContext(nc) as tc, ExitStack() as ctx:
        consts = ctx.enter_context(tc.tile_pool(name="consts", bufs=1))
        s_pool = ctx.enter_context(tc.tile_pool(name="s", bufs=bufs["s"]))
        u_pool = ctx.enter_context(tc.tile_pool(name="u", bufs=bufs["u"]))
        et_pool = ctx.enter_context(tc.tile_pool(name="et", bufs=bufs["et"]))
        o_pool = ctx.enter_context(tc.tile_pool(name="o", bufs=bufs["o"]))
        st_pool = ctx.enter_context(tc.tile_pool(name="stats", bufs=2))
        pst = ctx.enter_context(tc.tile_pool(name="pst", bufs=bufs["pst"], space="PSUM"))
        pso = ctx.enter_context(tc.tile_pool(name="pso", bufs=bufs["pso"], space="PSUM"))
        psr = ctx.enter_context(tc.tile_pool(name="psr", bufs=bufs["psr"], space="PSUM"))

        ident = consts.tile([P, P], BF16)
        make_identity(nc, ident)
        ones = consts.tile([P, 1], BF16)
        nc.vector.memset(ones[:], 1.0)

        if dma_only:
            o_fake = consts.tile([P, og, D], BF16)
            nc.vector.memset(o_fake[:], 0.0)

        loop_ctx = tc.For_i(0, repeat, 1) if repeat > 1 else None
        if loop_ctx is not None:
            ctx.enter_context(loop_ctx)

        NG = C // tg

        for b in range(BPC):
            s_halves = [
                S[b].rearrange("(h t) j -> h t j", h=NH)[h].rearrange(
                    "(p c) j -> p c j", c=CH
                )
                for h in range(NH)
            ]
            o_halves = [
                O[b].rearrange("(h t) d -> h t d", h=NH)[h].rearrange(
                    "(p c) d -> p c d", c=CH
                )
                for h in range(NH)
            ]

            def s_src_ap(a, bnd):
                h = a // CH
                return s_halves[h][:, a - h * CH : bnd - h * CH, :]

            def o_dst_ap(a, bnd):
                h = a // CH
                return o_halves[h][:, a - h * CH : bnd - h * CH, :]

            if dma_only:
                s_sb = s_pool.tile([P, C, J], BF16)
                for ss in range(s_split):
                    cs = C // s_split
                    in_eng.dma_start(
                        s_sb[:, ss * cs : (ss + 1) * cs, :],
                        s_src_ap(ss * cs, (ss + 1) * cs),
                    )
                u_sb = u_pool.tile([P, D], BF16)
                in_eng.dma_start(u_sb[:], U[b])
                for og_g in range(C // og):
                    out_eng(og_g).dma_start(
                        o_dst_ap(og_g * og, (og_g + 1) * og), o_fake[:]
                    )
                continue
            # --- loads ---
            s_sb = s_pool.tile([P, C, J], BF16)
            for ss in range(s_split):
                cs = C // s_split
                if not skip_in_dma:
                    in_eng.dma_start(
                        s_sb[:, ss * cs : (ss + 1) * cs, :],
                        s_src_ap(ss * cs, (ss + 1) * cs),
                    )
            if skip_in_dma:
                nc.vector.memset(s_sb[:, 0:1, :], 0.1)
            u_sb = u_pool.tile([P, D], BF16)
            if not skip_in_dma:
                in_eng.dma_start(u_sb[:], U[b])

            ps_r = psr.tile([P, C], F32, tag="ps_r", name=f"ps_r_{b}")
            rinv = st_pool.tile([P, C], F32, tag="rinv", name=f"rinv_{b}")

            et_sb = [None] * NG
            o_sb = [None] * (C // og)

            def do_transposes(g):
                # raw-S transposes (bf16 passthrough) into one PSUM bank,
                # then ONE act-exp copyback: E^T = exp(S^T) -> bf16 lhsT.
                et_ps = pst.tile([P, tg, P], BF16, tag="et_ps", name=f"et_ps_{b}_{g}")
                for k in range(tg):
                    nc.tensor.transpose(
                        et_ps[:, k, :], s_sb[:, g * tg + k, :], ident[:]
                    )
                et_sb[g] = et_pool.tile(
                    [P, tg, P], BF16, tag="et_sb", name=f"et_sb_{b}_{g}"
                )
                nc.scalar.activation(
                    et_sb[g][:], et_ps[:], mybir.ActivationFunctionType.Exp
                )

            def do_rmms(g):
                # softmax denominators via N=1 matmul against ones (PE
                # partition-reduce), one PSUM column per chunk; then recip.
                for k in range(tg):
                    c = g * tg + k
                    nc.tensor.matmul(
                        ps_r[:, c : c + 1], et_sb[g][:, k, :], ones[:],
                        start=True, stop=True,
                    )
                nc.vector.reciprocal(
                    rinv[:, g * tg : (g + 1) * tg],
                    ps_r[:, g * tg : (g + 1) * tg],
                )

            def do_matmul(c):
                o_ps = pso.tile([P, D], F32, tag="o_ps", name=f"o_ps_{b}_{c}")
                g, k = divmod(c, tg)
                nc.tensor.matmul(
                    o_ps[:], et_sb[g][:, k, :], u_sb[:], start=True, stop=True
                )
                og_g, gi = divmod(c, og)
                if gi == 0:
                    o_sb[og_g] = o_pool.tile(
                        [P, og, D], BF16, tag="o_sb", name=f"o_sb_{b}_{c}"
                    )
                if c in act_set:
                    nc.scalar.mul(o_sb[og_g][:, gi, :], o_ps[:], rinv[:, c : c + 1])
                else:
                    nc.vector.tensor_scalar_mul(
                        o_sb[og_g][:, gi, :], o_ps[:], rinv[:, c : c + 1]
                    )
                if gi == og - 1 and not skip_out_dma:
                    out_eng(og_g).dma_start(
                        o_dst_ap(og_g * og, (og_g + 1) * og), o_sb[og_g][:]
                    )

            # software pipeline: transposes run LOOKAHEAD groups ahead so the
            # act-exp latency hides behind PE matmuls of earlier groups.
            la = min(lookahead, NG)
            for g in range(la):
                do_transposes(g)
            for g in range(NG):
                if g + la < NG:
                    do_transposes(g + la)
                do_rmms(g)
                for k in range(tg):
                    do_matmul(g * tg + k)

    nc.compile()
    return nc


_NC_CACHE = None


def _get_nc():
    global _NC_CACHE
    if _NC_CACHE is None:
        _NC_CACHE = build_nc()
    return _NC_CACHE


def make_in_maps(U, S):
    U = np.asarray(U).astype(NP_BF16)
    S = np.asarray(S).astype(NP_BF16)
    return [
        {
            "S": np.ascontiguousarray(S[i * BPC : (i + 1) * BPC]),
            "U": np.ascontiguousarray(U[i * BPC : (i + 1) * BPC]),
        }
        for i in range(N_CORES)
    ]


def kernel(U, S):
    nc = _get_nc()
    in_maps = make_in_maps(U, S)
    try:
        res = run_bass_kernel_spmd(nc, in_maps, core_ids=list(range(N_CORES)))
    except Exception:
        # transient device/runtime hiccup: retry once
        res = run_bass_kernel_spmd(nc, in_maps, core_ids=list(range(N_CORES)))
    out = np.concatenate(
        [np.asarray(res.results[i]["O"]) for i in range(N_CORES)], axis=0
    ).astype(np.float32)
    return out
